# revision 1
# baseline (speedup 1.0000x reference)
"""Trainium2 Bass kernel for the convolutional differentiable-LUT-tree layer.

Per patch-row b, tree t: p = unfold(x) [2048, 400]; three LUT layers
(36/6/1 nodes per tree), each node = 64-entry multilinear LUT over 6 gathered
inputs, evaluated as 6 lerp levels:
  acc_{k+1}[r] = acc_k[r] + g_k * (acc_k[r+half] - acc_k[r]),  half = 32>>k

Mapping (8-way row shard; core c owns image c's 256 rows; fp16 on-chip):
  node-major: nodes on partitions, rows on free dim.
  - layer-0 input gather is host-side layout prep (g0 input tensor, DMA'd in
    3 chunks overlapped with compute)
  - level 0: ACT activation(Identity, scale=sig_hi-sig_lo, bias=sig_lo) --
    per-partition table constants, 32 ops/tile, never expanded over rows
  - levels 1-5: DVE in-place tensor_tensor chain (sub, mul-row-bcast, add),
    chained on an explicit semaphore (raw bass needs same-engine RAW sems)
  - h0/h1 staged to DRAM; layer-1/2 gathers via gpsimd indirect_dma_start
    (128 rows per call, per-partition offset tile)
"""

import numpy as np

B_IMG, C_IN, H, W = 8, 16, 32, 32
C_OUT = 64
KH = KW = 5
SH = SW = 2
PH = PW = 2
INPUT_SIZE = C_IN * KH * KW          # 400
OH = OW = 16
BP = B_IMG * OH * OW                 # 2048
N_CORES = 8
B_LOC = BP // N_CORES                # 256

L0_NODES, L1_NODES, L2_NODES = C_OUT * 36, C_OUT * 6, C_OUT
L0_TILES, L1_TILES = L0_NODES // 128, L1_NODES // 128        # 18, 3
N_TILES = L0_TILES + L1_TILES + 1                            # 22
G0_CHUNKS = 3
TILES_PER_CHUNK = L0_TILES // G0_CHUNKS                      # 6
L1_SLOTS = L1_TILES * 6                                      # 18
L2_SLOTS = 6

_CACHE = {}


def _unfold_np(x):
    xp = np.pad(x, ((0, 0), (0, 0), (PH, PH), (PW, PW)))
    ri = (np.arange(OH) * SH)[:, None] + np.arange(KH)[None, :]
    ci = (np.arange(OW) * SW)[:, None] + np.arange(KW)[None, :]
    p = xp[:, :, ri[:, None, :, None], ci[None, :, None, :]]
    p = np.transpose(p, (0, 2, 3, 1, 4, 5))
    return p.reshape(BP, INPUT_SIZE)


def _node_major(tbl, n_tiles):
    t = np.asarray(tbl, np.float32).reshape(-1, 64)
    pad = n_tiles * 128 - t.shape[0]
    if pad:
        t = np.concatenate([t, np.zeros((pad, 64), np.float32)], 0)
    return np.ascontiguousarray(t.reshape(n_tiles, 128, 64).transpose(1, 0, 2))


def _build_indices(idx0, idx1, idx2):
    idx0 = np.asarray(idx0).reshape(L0_NODES, 6)
    t_of_l1 = np.repeat(np.arange(C_OUT), 6)
    idx1g = np.asarray(idx1).reshape(L1_NODES, 6) + t_of_l1[:, None] * 36
    t_of_l2 = np.arange(C_OUT)
    idx2g = np.asarray(idx2).reshape(L2_NODES, 6) + t_of_l2[:, None] * 6

    # g0 gather order: slot = tile*6 + k, j = slot*128 + p, node = tile*128+p
    j0 = np.empty(L0_TILES * 6 * 128, np.int64)
    for tile in range(L0_TILES):
        for k in range(6):
            slot = tile * 6 + k
            j0[slot * 128:(slot + 1) * 128] = idx0[tile * 128:(tile + 1) * 128, k]
    # indirect-DMA offset tiles: [128, n_slots] int32
    gx1 = np.empty((128, L1_SLOTS), np.int32)
    for tile in range(L1_TILES):
        for k in range(6):
            gx1[:, tile * 6 + k] = idx1g[tile * 128:(tile + 1) * 128, k]
    gx2 = np.zeros((128, L2_SLOTS), np.int32)
    for k in range(6):
        gx2[0:64, k] = idx2g[:, k]
    return j0, np.ascontiguousarray(gx1), np.ascontiguousarray(gx2)


def _build_program():
    import concourse.bass as bass
    import concourse.mybir as mybir

    f16, f32, i32 = mybir.dt.float16, mybir.dt.float32, mybir.dt.int32
    AF = mybir.ActivationFunctionType

    nc = bass.Bass()

    g0in = nc.declare_dram_parameter("g0", [128, L0_TILES * 6, B_LOC], f16, isOutput=False)
    tbl0 = nc.declare_dram_parameter("tbl0", [128, L0_TILES, 64], f32, isOutput=False)
    tbl1 = nc.declare_dram_parameter("tbl1", [128, L1_TILES, 64], f32, isOutput=False)
    tbl2 = nc.declare_dram_parameter("tbl2", [128, 1, 64], f32, isOutput=False)
    gidx1 = nc.declare_dram_parameter("gidx1", [128, L1_SLOTS], i32, isOutput=False)
    gidx2 = nc.declare_dram_parameter("gidx2", [128, L2_SLOTS], i32, isOutput=False)
    out = nc.declare_dram_parameter("out", [C_OUT, B_LOC], f32, isOutput=True)

    h0d = nc.dram_tensor("h0d", [L0_NODES, B_LOC], f16)
    h1d = nc.dram_tensor("h1d", [L1_NODES, B_LOC], f16)

    layer_of = [0] * L0_TILES + [1] * L1_TILES + [2]

    from contextlib import ExitStack
    es = ExitStack()
    with es:
        sb = lambda *a: es.enter_context(nc.sbuf_tensor(*a))
        sem = lambda n: es.enter_context(nc.semaphore(n))
        g0sb = sb("g0sb", [128, L0_TILES * 6, B_LOC], f16)
        g1sb = sb("g1sb", [128, L1_SLOTS, B_LOC], f16)
        g2sb = sb("g2sb", [128, L2_SLOTS, B_LOC], f16)
        acc0 = sb("acc0", [128, 32, B_LOC], f16)
        acc1 = sb("acc1", [128, 32, B_LOC], f16)
        acc2 = sb("acc2", [128, 32, B_LOC], f16)
        h0sb = sb("h0sb", [128, L0_TILES, B_LOC], f16)
        h1sb = sb("h1sb", [128, L1_TILES, B_LOC], f16)
        h2sb = sb("h2sb", [64, B_LOC], f32)
        tb0 = sb("tb0", [128, L0_TILES, 64], f32)
        tb1 = sb("tb1", [128, L1_TILES, 64], f32)
        tb2 = sb("tb2", [128, 1, 64], f32)
        sg0 = sb("sg0", [128, L0_TILES, 64], f32)
        sg1 = sb("sg1", [128, L1_TILES, 64], f32)
        sg2 = sb("sg2", [128, 1, 64], f32)
        d0 = sb("d0", [128, L0_TILES, 32], f32)
        d1 = sb("d1", [128, L1_TILES, 32], f32)
        d2 = sb("d2", [128, 1, 32], f32)
        gx1 = sb("gx1", [128, L1_SLOTS], i32)
        gx2 = sb("gx2", [128, L2_SLOTS], i32)
        sINg = sem("sINg"); sINt = sem("sINt"); sSIG = sem("sSIG"); sD = sem("sD")
        sG0c = [sem("sG0a"), sem("sG0b"), sem("sG0cc")]
        sG1c = [sem("sG1a"), sem("sG1b"), sem("sG1cc")]
        sG2 = sem("sG2")
        sTS = sem("sTS"); sTT = sem("sTT"); sDVE = sem("sDVE")
        sH0D = sem("sH0D"); sH1D = sem("sH1D"); sOUT = sem("sOUT")
        block = es.enter_context(nc.Block())

        accs = [acc0, acc1, acc2]
        gsbs = [g0sb, g1sb, g2sb]
        sgs = [sg0, sg1, sg2]
        ds = [d0, d1, d2]
        tile_base = [0, L0_TILES, L0_TILES + L1_TILES]

        def g_wait(eng, T):
            lay = layer_of[T]
            if lay == 0:
                eng.wait_ge(sG0c[T // TILES_PER_CHUNK], 16)
            elif lay == 1:
                eng.wait_ge(sG1c[T - L0_TILES], 16 * 6)
            else:
                eng.wait_ge(sG2, 16 * L2_SLOTS)

        @block.sync
        def _(sync):
            sync.dma_start(out=gx1[:], in_=gidx1[:]).then_inc(sINg, 16)
            sync.dma_start(out=gx2[:], in_=gidx2[:]).then_inc(sINg, 16)
            sync.dma_start(out=tb0[:], in_=tbl0[:]).then_inc(sINt, 16)
            sync.dma_start(out=tb1[:], in_=tbl1[:]).then_inc(sINt, 16)
            sync.dma_start(out=tb2[:], in_=tbl2[:]).then_inc(sINt, 16)
            for c in range(G0_CHUNKS):
                s0 = c * TILES_PER_CHUNK * 6
                s1 = (c + 1) * TILES_PER_CHUNK * 6
                sync.dma_start(
                    out=g0sb[:, s0:s1, :], in_=g0in[:, s0:s1, :]
                ).then_inc(sG0c[c], 16)
            for T in range(L0_TILES):
                sync.wait_ge(sTT, T + 1)
                sync.dma_start(
                    out=h0d[T * 128:(T + 1) * 128, :], in_=h0sb[:, T, :]
                ).then_inc(sH0D, 16)
            sync.wait_ge(sTT, L0_TILES + L1_TILES)
            sync.dma_start(
                out=h1d[:].rearrange("(t p) b -> p t b", p=128), in_=h1sb[:]
            ).then_inc(sH1D, 16)
            sync.wait_ge(sTT, N_TILES)
            sync.dma_start(out=out[:], in_=h2sb[:]).then_inc(sOUT, 16)
            sync.wait_ge(sOUT, 16)

        @block.gpsimd
        def _(gp):
            gp.wait_ge(sINg, 32)
            gp.wait_ge(sH0D, 16 * L0_TILES)
            for t in range(L1_SLOTS):
                gp.indirect_dma_start(
                    out=g1sb[:, t, :], out_offset=None, in_=h0d[:],
                    in_offset=bass.IndirectOffsetOnAxis(ap=gx1[:, t:t + 1], axis=0),
                ).then_inc(sG1c[t // 6], 16)
            gp.wait_ge(sH1D, 16)
            for t in range(L2_SLOTS):
                gp.indirect_dma_start(
                    out=g2sb[:, t, :], out_offset=None, in_=h1d[:],
                    in_offset=bass.IndirectOffsetOnAxis(ap=gx2[:, t:t + 1], axis=0),
                ).then_inc(sG2, 16)

        @block.scalar
        def _(act):
            act.wait_ge(sINt, 48)
            act.activation(sg0[:], tb0[:], AF.Sigmoid)
            act.activation(sg1[:], tb1[:], AF.Sigmoid)
            act.activation(sg2[:], tb2[:], AF.Sigmoid).then_inc(sSIG, 1)
            act.wait_ge(sD, 1)
            for T in range(N_TILES):
                lay = layer_of[T]
                Tl = T - tile_base[lay]
                w = 64 if lay == 2 else 128
                acc, gsb, sg, d = accs[T % 3], gsbs[lay], sgs[lay], ds[lay]
                g_wait(act, T)
                if T >= 3:
                    act.wait_ge(sTT, T - 2)
                gin = gsb[0:w, Tl * 6, :] if lay < 2 else gsb[0:64, 0, :]
                for r in range(32):
                    ins = act.activation(
                        acc[0:w, r, :], gin, AF.Identity,
                        bias=sg[0:w, Tl, r:r + 1],
                        scale=d[0:w, Tl, r:r + 1],
                    )
                ins.then_inc(sTS, 1)

        @block.vector
        def _(dve):
            chain = {"n": 0}

            def vop(ins, final_sem=None):
                # raw-bass same-engine RAW needs explicit sems: chain every
                # DVE op on sDVE; group-final ops signal their real sem instead
                ins._wait_ge(sDVE, chain["n"])
                if final_sem is None:
                    ins.then_inc(sDVE, 1)
                    chain["n"] += 1
                else:
                    ins.then_inc(final_sem, 1)
                return ins

            dve.wait_ge(sSIG, 1)
            vop(dve.tensor_sub(d0[:, :, :], sg0[:, :, 32:64], sg0[:, :, 0:32]))
            vop(dve.tensor_sub(d1[:, :, :], sg1[:, :, 32:64], sg1[:, :, 0:32]))
            vop(dve.tensor_sub(d2[:, :, :], sg2[:, :, 32:64], sg2[:, :, 0:32]),
                final_sem=sD)
            for T in range(N_TILES):
                lay = layer_of[T]
                Tl = T - tile_base[lay]
                w = 64 if lay == 2 else 128
                acc, gsb = accs[T % 3], gsbs[lay]
                dve.wait_ge(sTS, T + 1)
                g_wait(dve, T)
                for k in range(1, 6):
                    half = 32 >> k
                    lo = acc[0:w, 0:half, :]
                    hi = acc[0:w, half:2 * half, :]
                    if lay < 2:
                        g = gsb[0:w, Tl * 6 + k, :]
                    else:
                        g = gsb[0:64, k, :]
                    gb = g.unsqueeze(1).broadcast_to([w, half, B_LOC])
                    vop(dve.tensor_sub(hi, hi, lo))
                    vop(dve.tensor_mul(hi, hi, gb))
                    if k < 5:
                        vop(dve.tensor_add(lo, lo, hi))
                    else:
                        if lay == 0:
                            dst = h0sb[:, Tl, :]
                        elif lay == 1:
                            dst = h1sb[:, Tl, :]
                        else:
                            dst = h2sb[:]
                        vop(dve.tensor_add(dst, acc[0:w, 0, :], acc[0:w, 1, :]),
                            final_sem=sTT)

    return nc


def _get_program():
    if "nc" not in _CACHE:
        _CACHE["nc"] = _build_program()
    return _CACHE["nc"]


def prepare_inputs(x, idx0, table0, idx1, table1, idx2, table2):
    p = _unfold_np(np.asarray(x, np.float32))             # [2048, 400]
    j0, gx1, gx2 = _build_indices(idx0, idx1, idx2)
    t0 = _node_major(table0, L0_TILES)
    t1 = _node_major(table1, L1_TILES)
    t2 = _node_major(table2, 1)
    in_maps = []
    for c in range(N_CORES):
        pcT = p[c * B_LOC:(c + 1) * B_LOC, :].T.astype(np.float16)  # [400, 256]
        g0 = np.ascontiguousarray(
            pcT[j0].reshape(L0_TILES * 6, 128, B_LOC).transpose(1, 0, 2)
        )
        in_maps.append({
            "g0": g0, "tbl0": t0, "tbl1": t1, "tbl2": t2,
            "gidx1": gx1, "gidx2": gx2,
        })
    return in_maps


def assemble_output(per_core_out):
    h2 = np.stack(per_core_out, 0)                        # [8, 64, 256]
    return np.ascontiguousarray(h2.reshape(B_IMG, C_OUT, OH, OW).astype(np.float32))


def kernel(x, idx0, table0, idx1, table1, idx2, table2):
    from concourse.bass_utils import run_bass_kernel_spmd

    nc = _get_program()
    in_maps = prepare_inputs(x, idx0, table0, idx1, table1, idx2, table2)
    res = run_bass_kernel_spmd(nc, in_maps, list(range(N_CORES)))
    outs = [np.asarray(res.results[c]["out"], np.float32) for c in range(N_CORES)]
    return assemble_output(outs)

